# revision 8
# baseline (speedup 1.0000x reference)
"""Trainium2 Bass kernel for nn_LIFcomplexLayer.

Computes: Wx = x @ W.T ; BatchNorm(train stats over (B,T)) ; complex-decay
LIF recurrence with spike output.

Sharding: data-parallel over batch B=32 across 8 cores (4 each). BN statistics
are all-reduced across cores with a tiny [128, 2*HC] collective.

The recurrence is run in second-order form (imag state eliminated):
  ur_{t+1} = a_r*(2*ur_t - s_t) - |a|^2*(ur_{t-1} - s_{t-1}) + e_t
  e_t = d_t - a_r*d_{t-1}   (d = BN'd drive, precomputed in bulk)
One fused custom DVE op computes q_t = a_r*(2*ur_t - s_t) from the raw
ur column per step; the |a|^2 term is folded into the next drive column by
the Pool engine with one step of slack, so the DVE critical path is only
2 instructions per time step.

Phases:
  A: stream x, PE-transpose 128x128 blocks, f32 matmuls -> Wx^T resident in
     SBUF laid out [h(128p), hc, b, t]; per-tile sums/sumsq partials.
  B: AllReduce stats; BN fold; in-place e-transform of the drive buffer.
  C: serial recurrence; raw ur overwrites the consumed drive column; spike
     thresholding + output DMA run chunk-by-chunk behind the recurrence.
"""

import sys

if "/opt/trn_rl_repo" not in sys.path:
    sys.path.insert(0, "/opt/trn_rl_repo")

import os
import numpy as np

B, T, I, H = 32, 2048, 512, 512
NCORES = 8
BLOC = B // NCORES          # 4 batches per core
P = 128                     # partitions
HC = H // P                 # 4 h-chunks
IC = I // P                 # 4 i-chunks
TC = 4                      # t-chunks per batch in phase A
TCH = T // TC               # 512 t per chunk
NTOT = float(B * T)         # BN sample count
SPK = 256                   # spike/output chunk (cols per chunk)

TSTEPS = int(os.environ.get("LIF_TSTEPS", str(T)))

_CACHE = {}


def _register_lif_ops():
    """Register the fused LIF custom DVE op at runtime (process-local)."""
    import concourse.dve_ops as dops
    from concourse.dve_spec import Spec, Src0, Src1, C2, lower, _has_src1
    from concourse.dve_uop import DveOpSpec

    if "LIF_Q_ANT" in dops._SUB_OPCODE_FOR_NAME:
        return (
            [o for o in dops.OPS if o.name == "LIF_Q_ANT"][0],
            [o for o in dops.OPS if o.name == "LIF_R_ANT"][0],
        )

    # q = ((Src0 + Src0) - (Src0 > C2)) * Src1   [Src1 = a_r broadcast]
    q_spec = Spec(
        body=((Src0 + Src0) - (Src0 > C2)) * Src1,
        reference=lambda in0, in1, s0, s1, imm2: (
            (in0 + in0) - (in0 > imm2).astype(np.float32)
        )
        * in1,
    )
    # r = ((Src0 > C2) - Src0) * Src1            [Src1 = |a|^2 broadcast]
    r_spec = Spec(
        body=((Src0 > C2) - Src0) * Src1,
        reference=lambda in0, in1, s0, s1, imm2: (
            (in0 > imm2).astype(np.float32) - in0
        )
        * in1,
    )
    ops = []
    for name, spec in (("LIF_Q_ANT", q_spec), ("LIF_R_ANT", r_spec)):
        row = max(dops._SUB_OPCODE_FOR_NAME.values()) + 1
        dops._SUB_OPCODE_FOR_NAME[name] = row
        shas = {}
        for ver in ("v3", "v4"):
            s = DveOpSpec(
                name=name, opcode=row, uops=lower(spec, ver=ver),
                rd1_en=_has_src1(spec),
            )
            shas[ver] = s.sha(ver)
        op = dops.DveOp(name, spec, subdim=False, uops_sha=shas)
        dops.OPS.append(op)
        dops.CUSTOM_DVE_SPECS[name] = spec
        ops.append(op)
    return tuple(ops)


def _build():
    import concourse.bass as bass
    import concourse.bacc as bacc
    import concourse.tile as tile
    from concourse import mybir

    lif_q, lif_r = _register_lif_ops()

    dt = mybir.dt
    f32 = dt.float32
    Alu = mybir.AluOpType
    Act = mybir.ActivationFunctionType

    from contextlib import ExitStack

    nc = bacc.Bacc(
        "TRN2", target_bir_lowering=False, debug=False, num_devices=NCORES
    )

    x_d = nc.dram_tensor("x", [BLOC, T, I], f32, kind="ExternalInput").ap()
    wt_d = nc.dram_tensor("wt", [I, H], f32, kind="ExternalInput").ap()
    ident_d = nc.dram_tensor("ident", [P, P], f32, kind="ExternalInput").ap()
    # arep: 0=a_r, 1=asq(|a|^2) broadcast over b
    arep_d = nc.dram_tensor("arep", [P, 2, HC * BLOC], f32, kind="ExternalInput").ap()
    # bgh: 0=gsc, 1=hof, 2=ngsc(-gsc), 3=hofe(hof*(1-a_r))  (host-computed BN fold)
    bgh_d = nc.dram_tensor("bgh", [P, 4, HC], f32, kind="ExternalInput").ap()
    # init: 0 = a_r*m0 + w0 (added to d_0), 1 = asq*negm0 (added to e_1)
    init_d = nc.dram_tensor("init", [P, 2, HC * BLOC], f32, kind="ExternalInput").ap()
    out_d = nc.dram_tensor("out", [HC, P, BLOC, T], f32, kind="ExternalOutput").ap()

    with tile.TileContext(nc) as tc, ExitStack() as ctx:
        consts = ctx.enter_context(tc.tile_pool(name="consts", bufs=1))
        big = ctx.enter_context(tc.tile_pool(name="big", bufs=1))
        xin = ctx.enter_context(tc.tile_pool(name="xin", bufs=3))
        xtp = ctx.enter_context(tc.tile_pool(name="xtp", bufs=2))
        ppool = ctx.enter_context(tc.tile_pool(name="psumT", bufs=4, space="PSUM"))
        mpool = ctx.enter_context(tc.tile_pool(name="psumM", bufs=2, space="PSUM"))
        scr = ctx.enter_context(tc.tile_pool(name="scr", bufs=4))
        etmp_p = ctx.enter_context(tc.tile_pool(name="etmp", bufs=2))

        wt_sb = consts.tile([P, IC, H], f32)
        nc.sync.dma_start(wt_sb[:], wt_d.rearrange("(ic p) h -> p ic h", p=P))
        ident_sb = consts.tile([P, P], f32)
        nc.sync.dma_start(ident_sb[:], ident_d[:])
        arep_sb = consts.tile([P, 2, HC * BLOC], f32)
        nc.sync.dma_start(arep_sb[:], arep_d[:])
        bgh_sb = consts.tile([P, 4, HC], f32)
        nc.sync.dma_start(bgh_sb[:], bgh_d[:])
        init_sb = consts.tile([P, 2, HC * BLOC], f32)
        nc.sync.dma_start(init_sb[:], init_d[:])

        # Drive/state buffer, free dims (hc, b, t). e_t overwritten by ur_{t+1}.
        wxbuf = big.tile([P, HC * BLOC, T], f32)

        # ---- phase A ----
        for b in range(BLOC):
            for tcix in range(TC):
                xr = xin.tile([P, TC, I], f32)  # [t(128p), tt, i]
                nc.sync.dma_start(
                    xr[:],
                    x_d[b, tcix * TCH : (tcix + 1) * TCH, :].rearrange(
                        "(tt p) i -> p tt i", p=P
                    ),
                )
                xt = xtp.tile([P, IC, TCH], f32)  # [i(128p), ic, t]
                for tt in range(TC):
                    for ic in range(IC):
                        pt = ppool.tile([P, P], f32)
                        nc.tensor.transpose(
                            pt[:], xr[:, tt, ic * P : (ic + 1) * P], ident_sb[:]
                        )
                        nc.vector.tensor_scalar(
                            xt[:, ic, tt * P : (tt + 1) * P], pt[:], 0.0, None,
                            op0=Alu.bypass,
                        )
                for hc in range(HC):
                    pm = mpool.tile([P, TCH], f32)
                    for ic in range(IC):
                        nc.tensor.matmul(
                            pm[:],
                            lhsT=wt_sb[:, ic, hc * P : (hc + 1) * P],
                            rhs=xt[:, ic, :],
                            start=(ic == 0),
                            stop=(ic == IC - 1),
                        )
                    dst = wxbuf[:, hc * BLOC + b, tcix * TCH : (tcix + 1) * TCH]
                    nc.scalar.activation(dst, pm[:], Act.Identity)

        # BN fold params are host-computed (bgh): 0=gsc, 1=hof, 2=ngsc, 3=hofe
        gsc = bgh_sb[:, 0, :]
        hof = bgh_sb[:, 1, :]
        ngsc = bgh_sb[:, 2, :]
        hofe = bgh_sb[:, 3, :]

        # ---- phase B': e-transform, in place over wxbuf ----
        # row (raw Wx) -> col0 = gsc*row0 + hof ; col[1:] = ngsc*(a_r*row[:-1]
        #   - row[1:]) + hofe   (= d_t - a_r*d_{t-1})
        arh = arep_sb[:, 0].rearrange("p (h b) -> p h b", b=BLOC)[:, :, 0]  # [P, HC] a_r
        for hc in range(HC):
            for b in range(BLOC):
                row = wxbuf[:, hc * BLOC + b, :]
                etmp = etmp_p.tile([P, T - 1], f32)
                nc.vector.scalar_tensor_tensor(
                    etmp[:], row[:, : T - 1], arh[:, hc : hc + 1], row[:, 1:],
                    op0=Alu.mult, op1=Alu.subtract,
                )
                nc.vector.tensor_scalar(
                    row[:, 0:1], row[:, 0:1],
                    gsc[:, hc : hc + 1], hof[:, hc : hc + 1],
                    op0=Alu.mult, op1=Alu.add,
                )
                nc.scalar.activation(
                    row[:, 1:], etmp[:], Act.Identity,
                    bias=hofe[:, hc : hc + 1], scale=ngsc[:, hc : hc + 1],
                )

        # first-step specials: col0 += (a_r*m0 + w0) ; col1 += asq*negm0
        nc.vector.tensor_tensor(
            wxbuf[:, :, 0], init_sb[:, 0], wxbuf[:, :, 0], op=Alu.add
        )
        nc.gpsimd.tensor_tensor(
            wxbuf[:, :, 1], init_sb[:, 1], wxbuf[:, :, 1], op=Alu.add
        )

        # ---- phase C: serial recurrence (second-order form) ----
        # slot tau holds ur_{tau+1} once processed. At step t (1..T-1):
        #   q_t = a_r*(2*ur_t - s_t)        [DVE, fused custom op]
        #   col[t] = q_t + col[t]           [DVE; col[t] = e_t - asq*m_{t-1}]
        #   negm_t = s_t - ur_t             [Pool]
        #   r_t = asq*negm_t                [Pool]
        #   col[t+1] += r_t                 [Pool, one step of slack]
        arr3 = arep_sb[:, 0]   # [P, HC*BLOC] a_r
        asq3 = arep_sb[:, 1]   # [P, HC*BLOC] |a|^2
        spiked = 0

        def spike_and_out(c0, c1):
            # threshold cols [c0, c1) in place and DMA them out
            nc.vector.tensor_scalar(
                wxbuf[:, :, c0:c1], wxbuf[:, :, c0:c1], 0.5, None,
                op0=Alu.is_gt,
            )
            for hc in range(HC):
                nc.sync.dma_start(
                    out_d[hc, :, :, c0:c1],
                    wxbuf[:, hc * BLOC : (hc + 1) * BLOC, c0:c1],
                )

        for t in range(1, TSTEPS):
            pcol = wxbuf[:, :, t - 1]
            col = wxbuf[:, :, t]
            if t < TSTEPS - 1:
                r = scr.tile([P, HC * BLOC], f32)
                ncol = wxbuf[:, :, t + 1]
                nc.vector._custom_dve(
                    lif_r, out=r[:], in0=pcol, in1=asq3, imm2=0.5
                )
                nc.gpsimd.tensor_tensor(ncol, r[:], ncol, op=Alu.add)
            q = scr.tile([P, HC * BLOC], f32)
            nc.vector._custom_dve(
                lif_q, out=q[:], in0=pcol, in1=arr3, imm2=0.5
            )
            nc.vector.tensor_tensor(col, q[:], col, op=Alu.add)
            # chunked spike + output DMA, two columns behind the head
            if t == spiked + SPK + 1 and TSTEPS == T:
                spike_and_out(spiked, spiked + SPK)
                spiked += SPK

        # tail: spike + DMA whatever remains
        if TSTEPS == T:
            if spiked < T:
                spike_and_out(spiked, T)
        else:
            spike_and_out(0, TSTEPS)

    nc.compile()
    return nc


def _prep_host(x, W, log_log_alpha, log_dt, alpha_img, b, gamma, beta,
               u0_real, u0_imag, s0):
    lla = np.exp(log_log_alpha.astype(np.float64))
    dtv = np.exp(log_dt.astype(np.float64))
    z = (-lla + 1j * alpha_img.astype(np.float64)) * dtv
    alpha = np.exp(z)
    a_r = alpha.real.astype(np.float32)
    a_i = alpha.imag.astype(np.float32)
    asq = (alpha.real**2 + alpha.imag**2).astype(np.float32)

    wt = np.ascontiguousarray(W.T.astype(np.float32))  # [I, H]
    ident = np.eye(P, dtype=np.float32)

    def tohc(v):  # [H] -> [P, HC]
        return np.ascontiguousarray(v.reshape(HC, P).T.astype(np.float32))

    arep = np.zeros((P, 2, HC, BLOC), np.float32)
    arep[:, 0] = tohc(a_r)[:, :, None]
    arep[:, 1] = tohc(asq)[:, :, None]
    arep = arep.reshape(P, 2, HC * BLOC)

    # BN statistics on host (f32 matmul, f64 stats), folded with b
    Wx = x.reshape(-1, I).astype(np.float32) @ wt
    mean = Wx.astype(np.float64).mean(0)
    var = Wx.astype(np.float64).var(0)
    inv = 1.0 / np.sqrt(var + 1e-5)
    bg = (b * gamma).astype(np.float64)
    gsc = (bg * inv)
    hof = (b * beta).astype(np.float64) - mean * gsc
    bgh = np.zeros((P, 4, HC), np.float32)
    bgh[:, 0] = tohc(gsc.astype(np.float32))
    bgh[:, 1] = tohc(hof.astype(np.float32))
    bgh[:, 2] = tohc((-gsc).astype(np.float32))
    bgh[:, 3] = tohc((hof * (1.0 - a_r.astype(np.float64))).astype(np.float32))

    # per-core init tensors
    m0 = (u0_real.astype(np.float32) - s0.astype(np.float32))  # [B, H]
    w0 = (-a_i[None, :] * u0_imag.astype(np.float32))
    c0add = (a_r[None, :] * m0 + w0).astype(np.float32)        # -> col0
    r0 = (asq[None, :] * (-m0)).astype(np.float32)             # -> col1
    return wt, ident, arep, bgh, c0add, r0


def kernel(x, W, log_log_alpha, log_dt, alpha_img, b, gamma, beta,
           u0_real, u0_imag, s0):
    from concourse.bass_utils import run_bass_kernel_spmd

    if "nc" not in _CACHE:
        _CACHE["nc"] = _build()
    nc = _CACHE["nc"]

    wt, ident, arep, bgh, c0add, r0 = _prep_host(
        x, W, log_log_alpha, log_dt, alpha_img, b, gamma, beta,
        u0_real, u0_imag, s0
    )

    def tocore(v, bs):  # [Bloc, H] -> [P, HC, BLOC]
        return np.ascontiguousarray(
            v[bs].T.reshape(HC, P, BLOC).transpose(1, 0, 2).astype(np.float32)
        )

    in_maps = []
    for c in range(NCORES):
        bs = slice(c * BLOC, (c + 1) * BLOC)
        init = np.zeros((P, 2, HC, BLOC), np.float32)
        init[:, 0] = tocore(c0add, bs)
        init[:, 1] = tocore(r0, bs)
        init = init.reshape(P, 2, HC * BLOC)
        in_maps.append({
            "x": np.ascontiguousarray(x[bs].astype(np.float32)),
            "wt": wt,
            "ident": ident,
            "arep": arep,
            "bgh": bgh,
            "init": init,
        })

    res = run_bass_kernel_spmd(
        nc,
        in_maps,
        core_ids=list(range(NCORES)),
        trace=bool(int(os.environ.get("LIF_TRACE", "0"))),
    )
    _CACHE["last_res"] = res
    out = np.empty((B, T, H), np.float32)
    for c in range(NCORES):
        o = res.results[c]["out"]  # [HC, P, BLOC, T]
        out[c * BLOC : (c + 1) * BLOC] = o.transpose(2, 3, 0, 1).reshape(
            BLOC, T, H
        )
    return out


# revision 9
# speedup vs baseline: 1.0361x; 1.0361x over previous
"""Trainium2 Bass kernel for nn_LIFcomplexLayer.

Computes: Wx = x @ W.T ; BatchNorm(train stats over (B,T)) ; complex-decay
LIF recurrence with spike output.

Sharding: data-parallel over batch B=32 across 8 cores (4 each). BN statistics
are all-reduced across cores with a tiny [128, 2*HC] collective.

The recurrence is run in second-order form (imag state eliminated):
  ur_{t+1} = a_r*(2*ur_t - s_t) - |a|^2*(ur_{t-1} - s_{t-1}) + e_t
  e_t = d_t - a_r*d_{t-1}   (d = BN'd drive, precomputed in bulk)
One fused custom DVE op computes q_t = a_r*(2*ur_t - s_t) from the raw
ur column per step; the |a|^2 term is folded into the next drive column by
the Pool engine with one step of slack, so the DVE critical path is only
2 instructions per time step.

Phases:
  A: stream x, PE-transpose 128x128 blocks, f32 matmuls -> Wx^T resident in
     SBUF laid out [h(128p), hc, b, t]; per-tile sums/sumsq partials.
  B: AllReduce stats; BN fold; in-place e-transform of the drive buffer.
  C: serial recurrence; raw ur overwrites the consumed drive column; spike
     thresholding + output DMA run chunk-by-chunk behind the recurrence.
"""

import sys

if "/opt/trn_rl_repo" not in sys.path:
    sys.path.insert(0, "/opt/trn_rl_repo")

import os
import numpy as np

B, T, I, H = 32, 2048, 512, 512
NCORES = 8
BLOC = B // NCORES          # 4 batches per core
P = 128                     # partitions
HC = H // P                 # 4 h-chunks
IC = I // P                 # 4 i-chunks
TC = 4                      # t-chunks per batch in phase A
TCH = T // TC               # 512 t per chunk
NTOT = float(B * T)         # BN sample count
SPK = 256                   # spike/output chunk (cols per chunk)

TSTEPS = int(os.environ.get("LIF_TSTEPS", str(T)))

_CACHE = {}


def _register_lif_ops():
    """Register the fused LIF custom DVE op at runtime (process-local)."""
    import concourse.dve_ops as dops
    from concourse.dve_spec import Spec, Src0, Src1, C2, lower, _has_src1
    from concourse.dve_uop import DveOpSpec

    if "LIF_Q_ANT" in dops._SUB_OPCODE_FOR_NAME:
        return (
            [o for o in dops.OPS if o.name == "LIF_Q_ANT"][0],
            [o for o in dops.OPS if o.name == "LIF_R_ANT"][0],
        )

    # q = ((Src0 + Src0) - (Src0 > C2)) * Src1   [Src1 = a_r broadcast]
    q_spec = Spec(
        body=((Src0 + Src0) - (Src0 > C2)) * Src1,
        reference=lambda in0, in1, s0, s1, imm2: (
            (in0 + in0) - (in0 > imm2).astype(np.float32)
        )
        * in1,
    )
    # r = ((Src0 > C2) - Src0) * Src1            [Src1 = |a|^2 broadcast]
    r_spec = Spec(
        body=((Src0 > C2) - Src0) * Src1,
        reference=lambda in0, in1, s0, s1, imm2: (
            (in0 > imm2).astype(np.float32) - in0
        )
        * in1,
    )
    ops = []
    for name, spec in (("LIF_Q_ANT", q_spec), ("LIF_R_ANT", r_spec)):
        row = max(dops._SUB_OPCODE_FOR_NAME.values()) + 1
        dops._SUB_OPCODE_FOR_NAME[name] = row
        shas = {}
        for ver in ("v3", "v4"):
            s = DveOpSpec(
                name=name, opcode=row, uops=lower(spec, ver=ver),
                rd1_en=_has_src1(spec),
            )
            shas[ver] = s.sha(ver)
        op = dops.DveOp(name, spec, subdim=False, uops_sha=shas)
        dops.OPS.append(op)
        dops.CUSTOM_DVE_SPECS[name] = spec
        ops.append(op)
    return tuple(ops)


def _build():
    import concourse.bass as bass
    import concourse.bacc as bacc
    import concourse.tile as tile
    from concourse import mybir

    lif_q, lif_r = _register_lif_ops()

    dt = mybir.dt
    f32 = dt.float32
    Alu = mybir.AluOpType
    Act = mybir.ActivationFunctionType

    from contextlib import ExitStack

    nc = bacc.Bacc(
        "TRN2", target_bir_lowering=False, debug=False, num_devices=NCORES
    )

    x_d = nc.dram_tensor("x", [BLOC, T, I], f32, kind="ExternalInput").ap()
    wt_d = nc.dram_tensor("wt", [I, H], f32, kind="ExternalInput").ap()
    ident_d = nc.dram_tensor("ident", [P, P], f32, kind="ExternalInput").ap()
    # arep: 0=a_r, 1=asq(|a|^2) broadcast over b
    arep_d = nc.dram_tensor("arep", [P, 2, HC * BLOC], f32, kind="ExternalInput").ap()
    # bgh: 0=b*gamma, 1=b*beta, 2=(1-a_r)
    bgh_d = nc.dram_tensor("bgh", [P, 3, HC], f32, kind="ExternalInput").ap()
    # init: 0 = a_r*m0 + w0 (added to d_0), 1 = asq*negm0 (added to e_1)
    init_d = nc.dram_tensor("init", [P, 2, HC * BLOC], f32, kind="ExternalInput").ap()
    out_d = nc.dram_tensor("out", [HC, P, BLOC, T], f32, kind="ExternalOutput").ap()

    with tile.TileContext(nc) as tc, ExitStack() as ctx:
        consts = ctx.enter_context(tc.tile_pool(name="consts", bufs=1))
        big = ctx.enter_context(tc.tile_pool(name="big", bufs=1))
        xin = ctx.enter_context(tc.tile_pool(name="xin", bufs=3))
        xtp = ctx.enter_context(tc.tile_pool(name="xtp", bufs=2))
        ppool = ctx.enter_context(tc.tile_pool(name="psumT", bufs=4, space="PSUM"))
        mpool = ctx.enter_context(tc.tile_pool(name="psumM", bufs=2, space="PSUM"))
        trash_p = ctx.enter_context(tc.tile_pool(name="trash", bufs=2))
        small = ctx.enter_context(tc.tile_pool(name="small", bufs=1))
        scr = ctx.enter_context(tc.tile_pool(name="scr", bufs=4))
        etmp_p = ctx.enter_context(tc.tile_pool(name="etmp", bufs=2))
        dram = ctx.enter_context(tc.tile_pool(name="dram", bufs=1, space="DRAM"))

        wt_sb = consts.tile([P, IC, H], f32)
        nc.sync.dma_start(wt_sb[:], wt_d.rearrange("(ic p) h -> p ic h", p=P))
        ident_sb = consts.tile([P, P], f32)
        nc.sync.dma_start(ident_sb[:], ident_d[:])
        arep_sb = consts.tile([P, 2, HC * BLOC], f32)
        nc.sync.dma_start(arep_sb[:], arep_d[:])
        bgh_sb = consts.tile([P, 3, HC], f32)
        nc.sync.dma_start(bgh_sb[:], bgh_d[:])
        init_sb = consts.tile([P, 2, HC * BLOC], f32)
        nc.sync.dma_start(init_sb[:], init_d[:])

        # Drive/state buffer, free dims (hc, b, t). e_t overwritten by ur_{t+1}.
        wxbuf = big.tile([P, HC * BLOC, T], f32)
        sumS = small.tile([P, HC, BLOC * TC], f32)
        sumQ = small.tile([P, HC, BLOC * TC], f32)

        # ---- phase A ----
        for b in range(BLOC):
            for tcix in range(TC):
                xr = xin.tile([P, TC, I], f32)  # [t(128p), tt, i]
                nc.sync.dma_start(
                    xr[:],
                    x_d[b, tcix * TCH : (tcix + 1) * TCH, :].rearrange(
                        "(tt p) i -> p tt i", p=P
                    ),
                )
                xt = xtp.tile([P, IC, TCH], f32)  # [i(128p), ic, t]
                for tt in range(TC):
                    for ic in range(IC):
                        pt = ppool.tile([P, P], f32)
                        nc.tensor.transpose(
                            pt[:], xr[:, tt, ic * P : (ic + 1) * P], ident_sb[:]
                        )
                        nc.vector.tensor_scalar(
                            xt[:, ic, tt * P : (tt + 1) * P], pt[:], 0.0, None,
                            op0=Alu.bypass,
                        )
                idx = b * TC + tcix
                for hc in range(HC):
                    pm = mpool.tile([P, TCH], f32)
                    for ic in range(IC):
                        nc.tensor.matmul(
                            pm[:],
                            lhsT=wt_sb[:, ic, hc * P : (hc + 1) * P],
                            rhs=xt[:, ic, :],
                            start=(ic == 0),
                            stop=(ic == IC - 1),
                        )
                    dst = wxbuf[:, hc * BLOC + b, tcix * TCH : (tcix + 1) * TCH]
                    nc.scalar.activation(
                        dst, pm[:], Act.Identity, accum_out=sumS[:, hc, idx : idx + 1]
                    )
                    trash = trash_p.tile([P, TCH], f32)
                    nc.vector.scalar_tensor_tensor(
                        trash[:],
                        dst,
                        1.0,
                        dst,
                        op0=Alu.bypass,
                        op1=Alu.mult,
                        accum_out=sumQ[:, hc, idx : idx + 1],
                    )

        # ---- phase B: stats all-reduce + BN fold ----
        stats = small.tile([P, 2, HC], f32)
        nc.vector.tensor_reduce(
            stats[:, 0, :], sumS[:], axis=mybir.AxisListType.X, op=Alu.add
        )
        nc.vector.tensor_reduce(
            stats[:, 1, :], sumQ[:], axis=mybir.AxisListType.X, op=Alu.add
        )
        cc_in = dram.tile([P, 2 * HC], f32)
        cc_out = dram.tile([P, 2 * HC], f32)
        nc.sync.dma_start(cc_in[:], stats[:].rearrange("p a h -> p (a h)"))
        nc.gpsimd.collective_compute(
            "AllReduce",
            Alu.add,
            replica_groups=[list(range(NCORES))],
            ins=[cc_in.opt()],
            outs=[cc_out.opt()],
        )
        gstats = small.tile([P, 2, HC], f32)
        nc.sync.dma_start(gstats[:], cc_out[:].rearrange("p (a h) -> p a h", a=2))

        mean = small.tile([P, HC], f32)
        ex2 = small.tile([P, HC], f32)
        var = small.tile([P, HC], f32)
        inv = small.tile([P, HC], f32)
        gsc = small.tile([P, HC], f32)
        ngsc = small.tile([P, HC], f32)
        hof = small.tile([P, HC], f32)
        hofe = small.tile([P, HC], f32)
        tmp = small.tile([P, HC], f32)
        nc.vector.tensor_scalar(mean[:], gstats[:, 0, :], 1.0 / NTOT, None, op0=Alu.mult)
        nc.vector.tensor_scalar(ex2[:], gstats[:, 1, :], 1.0 / NTOT, None, op0=Alu.mult)
        nc.vector.tensor_tensor(tmp[:], mean[:], mean[:], op=Alu.mult)
        nc.vector.tensor_tensor(var[:], ex2[:], tmp[:], op=Alu.subtract)
        nc.vector.tensor_scalar(var[:], var[:], 1e-5, None, op0=Alu.add)
        nc.scalar.sqrt(tmp[:], var[:])
        nc.vector.reciprocal(inv[:], tmp[:])
        nc.vector.tensor_tensor(gsc[:], bgh_sb[:, 0, :], inv[:], op=Alu.mult)
        nc.vector.tensor_scalar(ngsc[:], gsc[:], -1.0, None, op0=Alu.mult)
        nc.vector.tensor_tensor(tmp[:], mean[:], gsc[:], op=Alu.mult)
        nc.vector.tensor_tensor(hof[:], bgh_sb[:, 1, :], tmp[:], op=Alu.subtract)
        nc.vector.tensor_tensor(hofe[:], hof[:], bgh_sb[:, 2, :], op=Alu.mult)

        # ---- phase B': e-transform, in place over wxbuf ----
        # row (raw Wx) -> col0 = gsc*row0 + hof ; col[1:] = ngsc*(a_r*row[:-1]
        #   - row[1:]) + hofe   (= d_t - a_r*d_{t-1})
        arh = arep_sb[:, 0].rearrange("p (h b) -> p h b", b=BLOC)[:, :, 0]  # [P, HC] a_r
        for hc in range(HC):
            for b in range(BLOC):
                row = wxbuf[:, hc * BLOC + b, :]
                etmp = etmp_p.tile([P, T - 1], f32)
                nc.vector.scalar_tensor_tensor(
                    etmp[:], row[:, : T - 1], arh[:, hc : hc + 1], row[:, 1:],
                    op0=Alu.mult, op1=Alu.subtract,
                )
                nc.vector.tensor_scalar(
                    row[:, 0:1], row[:, 0:1],
                    gsc[:, hc : hc + 1], hof[:, hc : hc + 1],
                    op0=Alu.mult, op1=Alu.add,
                )
                nc.scalar.activation(
                    row[:, 1:], etmp[:], Act.Identity,
                    bias=hofe[:, hc : hc + 1], scale=ngsc[:, hc : hc + 1],
                )

        # first-step specials: col0 += (a_r*m0 + w0) ; col1 += asq*negm0
        nc.vector.tensor_tensor(
            wxbuf[:, :, 0], init_sb[:, 0], wxbuf[:, :, 0], op=Alu.add
        )
        nc.gpsimd.tensor_tensor(
            wxbuf[:, :, 1], init_sb[:, 1], wxbuf[:, :, 1], op=Alu.add
        )

        # ---- phase C: serial recurrence (second-order form) ----
        # slot tau holds ur_{tau+1} once processed. At step t (1..T-1):
        #   q_t = a_r*(2*ur_t - s_t)        [DVE, fused custom op]
        #   col[t] = q_t + col[t]           [DVE; col[t] = e_t - asq*m_{t-1}]
        #   negm_t = s_t - ur_t             [Pool]
        #   r_t = asq*negm_t                [Pool]
        #   col[t+1] += r_t                 [Pool, one step of slack]
        arr3 = arep_sb[:, 0]   # [P, HC*BLOC] a_r
        asq3 = arep_sb[:, 1]   # [P, HC*BLOC] |a|^2
        spiked = 0

        def spike_and_out(c0, c1):
            # threshold cols [c0, c1) in place and DMA them out
            nc.vector.tensor_scalar(
                wxbuf[:, :, c0:c1], wxbuf[:, :, c0:c1], 0.5, None,
                op0=Alu.is_gt,
            )
            for hc in range(HC):
                nc.sync.dma_start(
                    out_d[hc, :, :, c0:c1],
                    wxbuf[:, hc * BLOC : (hc + 1) * BLOC, c0:c1],
                )

        for t in range(1, TSTEPS):
            pcol = wxbuf[:, :, t - 1]
            col = wxbuf[:, :, t]
            q = scr.tile([P, HC * BLOC], f32)
            nc.vector._custom_dve(
                lif_q, out=q[:], in0=pcol, in1=arr3, imm2=0.5
            )
            nc.vector.tensor_tensor(col, q[:], col, op=Alu.add)
            if t < TSTEPS - 1:
                r = scr.tile([P, HC * BLOC], f32)
                ncol = wxbuf[:, :, t + 1]
                nc.vector._custom_dve(
                    lif_r, out=r[:], in0=pcol, in1=asq3, imm2=0.5
                )
                nc.gpsimd.tensor_tensor(ncol, r[:], ncol, op=Alu.add)
            # chunked spike + output DMA, two columns behind the head
            if t == spiked + SPK + 1 and TSTEPS == T:
                spike_and_out(spiked, spiked + SPK)
                spiked += SPK

        # tail: spike + DMA whatever remains
        if TSTEPS == T:
            if spiked < T:
                spike_and_out(spiked, T)
        else:
            spike_and_out(0, TSTEPS)

    nc.compile()
    return nc


def _prep_host(W, log_log_alpha, log_dt, alpha_img, b, gamma, beta,
               u0_real, u0_imag, s0):
    lla = np.exp(log_log_alpha.astype(np.float64))
    dtv = np.exp(log_dt.astype(np.float64))
    z = (-lla + 1j * alpha_img.astype(np.float64)) * dtv
    alpha = np.exp(z)
    a_r = alpha.real.astype(np.float32)
    a_i = alpha.imag.astype(np.float32)
    asq = (alpha.real**2 + alpha.imag**2).astype(np.float32)

    wt = np.ascontiguousarray(W.T.astype(np.float32))  # [I, H]
    ident = np.eye(P, dtype=np.float32)

    def tohc(v):  # [H] -> [P, HC]
        return np.ascontiguousarray(v.reshape(HC, P).T.astype(np.float32))

    arep = np.zeros((P, 2, HC, BLOC), np.float32)
    arep[:, 0] = tohc(a_r)[:, :, None]
    arep[:, 1] = tohc(asq)[:, :, None]
    arep = arep.reshape(P, 2, HC * BLOC)

    bgh = np.zeros((P, 3, HC), np.float32)
    bgh[:, 0] = tohc((b * gamma).astype(np.float32))
    bgh[:, 1] = tohc((b * beta).astype(np.float32))
    bgh[:, 2] = tohc((1.0 - a_r).astype(np.float32))

    # per-core init tensors
    m0 = (u0_real.astype(np.float32) - s0.astype(np.float32))  # [B, H]
    w0 = (-a_i[None, :] * u0_imag.astype(np.float32))
    c0add = (a_r[None, :] * m0 + w0).astype(np.float32)        # -> col0
    r0 = (asq[None, :] * (-m0)).astype(np.float32)             # -> col1
    return wt, ident, arep, bgh, c0add, r0


def kernel(x, W, log_log_alpha, log_dt, alpha_img, b, gamma, beta,
           u0_real, u0_imag, s0):
    from concourse.bass_utils import run_bass_kernel_spmd

    if "nc" not in _CACHE:
        _CACHE["nc"] = _build()
    nc = _CACHE["nc"]

    wt, ident, arep, bgh, c0add, r0 = _prep_host(
        W, log_log_alpha, log_dt, alpha_img, b, gamma, beta,
        u0_real, u0_imag, s0
    )

    def tocore(v, bs):  # [Bloc, H] -> [P, HC, BLOC]
        return np.ascontiguousarray(
            v[bs].T.reshape(HC, P, BLOC).transpose(1, 0, 2).astype(np.float32)
        )

    in_maps = []
    for c in range(NCORES):
        bs = slice(c * BLOC, (c + 1) * BLOC)
        init = np.zeros((P, 2, HC, BLOC), np.float32)
        init[:, 0] = tocore(c0add, bs)
        init[:, 1] = tocore(r0, bs)
        init = init.reshape(P, 2, HC * BLOC)
        in_maps.append({
            "x": np.ascontiguousarray(x[bs].astype(np.float32)),
            "wt": wt,
            "ident": ident,
            "arep": arep,
            "bgh": bgh,
            "init": init,
        })

    res = run_bass_kernel_spmd(
        nc,
        in_maps,
        core_ids=list(range(NCORES)),
        trace=bool(int(os.environ.get("LIF_TRACE", "0"))),
    )
    _CACHE["last_res"] = res
    out = np.empty((B, T, H), np.float32)
    for c in range(NCORES):
        o = res.results[c]["out"]  # [HC, P, BLOC, T]
        out[c * BLOC : (c + 1) * BLOC] = o.transpose(2, 3, 0, 1).reshape(
            BLOC, T, H
        )
    return out
